# revision 28
# baseline (speedup 1.0000x reference)
"""Relative-position attention (Music-Transformer style skew) + LayerNorm,
distributed over 8 TRN2 NeuronCores.

Sharding: data-parallel over batch (B=4) x tensor-parallel over head-halves
(H=8 -> 2 groups of 4). Core c handles batch b=c//2, heads [4*(c%2), 4*(c%2)+4),
producing output channels [256*(c%2), +256) of y[b]. The final LayerNorm needs
full-E stats, exchanged via a tiny pairwise AllReduce of (sum, sumsq).

Skew trick: Srel[i,j] = F[(i+1)*S + j] where F is the row-major flat view of
the padded matrix P[i, 0]=0, P[i, 1+l]=QEr[i, l] (P is [S, S+1]). We bounce P
through DRAM in fp8e4m3; the skewed read back is a plain strided DMA.

Perf notes vs the old fp32r version:
 - All PE-streamed data is fp16 (same PE rate as bf16, ~4x faster than
   fp32r at N=512, FWL on weight loads). PSUM accumulation stays fp32.
 - Heads are processed in row-tiled pairs: head 2p lives in PE rows 0-63,
   head 2p+1 in rows 64-127 (qT/kT/erT halves), so the K=64 QEr/QK matmuls
   of the two heads execute concurrently in the array.
 - The QEr bounce is fp8 (2x less HBM traffic than bf16); measured numpy
   end-to-end error ~6e-3 vs the 2e-2 budget.
 - Elementwise work (PSUM evacuations, score adds) is split between ACT
   and DVE to balance engine busy time; exp runs on ACT from a 2-bank
   PSUM tile in one [128, S] activation per head-block.
"""

import numpy as np

import concourse.bass as bass
import concourse.mybir as mybir
from concourse import masks
from concourse.tile import TileContext

F32 = mybir.dt.float32
F16 = mybir.dt.float16
FP8 = mybir.dt.float8e4

B, S, E, H = 4, 2048, 512, 8
HD = E // H          # 64
HLOC = 4             # heads per core
CH = HLOC * HD       # 256 output channels per core
SCALE = float(E) ** -0.5
EPS = 1e-5
N_CORES = 8
# Srel add strategy: True = SWDGE DMA-accumulate (fp8 DRAM read casts and
# adds into the fp16 score tile in the SDMA datapath, freeing DVE); False
# = DVE tensor_tensor add of (psum, fp8 srel tile).
DMA_ACCUM_SREL = False


def build_nc(s=S, n_cores=N_CORES, debug=False, legalize=True):
    """Build the per-core Bass graph (SPMD: same graph on all cores)."""
    nc = bass.Bass(target_bir_lowering=False, debug=debug)

    SB = s // 128        # number of 128-row blocks
    KC = s // 512        # number of 512-col chunks
    NTH = s // 128       # transpose blocks per scores row-block
    NPAIR = HLOC // 2    # head pairs per core
    # evac/add chunking: pieces of <=1024 cols
    CW = min(1024, s)
    NCH = s // CW
    # transpose/exp half-granularity: [128, CW] fp16 = one PSUM bank
    NHALF = s // CW
    NTH2 = CW // 128     # transposes per half

    x_d = nc.declare_dram_parameter("x", [s, E], F32, isOutput=False)
    wq_d = nc.declare_dram_parameter("wq", [CH, E], F32, isOutput=False)
    wk_d = nc.declare_dram_parameter("wk", [CH, E], F32, isOutput=False)
    wv_d = nc.declare_dram_parameter("wv", [CH, E], F32, isOutput=False)
    er_d = nc.declare_dram_parameter("er", [s, HD], F32, isOutput=False)
    gamma_d = nc.declare_dram_parameter("gamma", [1, CH], F32, isOutput=False)
    beta_d = nc.declare_dram_parameter("beta", [1, CH], F32, isOutput=False)
    out_d = nc.declare_dram_parameter("out", [s, CH], F32, isOutput=True)

    # Padded-QEr bounce buffers, one per head, flat [S*(S+1)] fp8.
    p_d = nc.dram_tensor("pbuf", [HLOC, s * (s + 1)], FP8)
    cc_in = nc.dram_tensor("cc_in", [s, 2], F32)
    cc_out = nc.dram_tensor("cc_out", [s, 2], F32)

    pairs = [[2 * i, 2 * i + 1] for i in range(n_cores // 2)]

    with TileContext(nc) as tc:
        with (
            tc.tile_pool(name="const", bufs=1) as const_pool,
            tc.tile_pool(name="persist", bufs=1) as pp,
        ):
            ident_f32 = const_pool.tile([128, 128], F32)
            ident_f16 = const_pool.tile([128, 128], F16)
            masks.make_identity(nc, ident_f32[:])
            masks.make_identity(nc, ident_f16[:])
            gamma_bc = const_pool.tile([128, CH], F32)
            beta_bc = const_pool.tile([128, CH], F32)
            eps_t = const_pool.tile([128, 1], F32)
            nc.gpsimd.memset(eps_t[:], EPS)
            nc.sync.dma_start(gamma_bc[:], gamma_d[:].broadcast_to((128, CH)))
            nc.sync.dma_start(beta_bc[:], beta_d[:].broadcast_to((128, CH)))

            # ---- persistent SBUF tensors (fp16 for PE streaming) ----
            xT = [pp.tile([128, s], F16, tag=f"xT{ec}", name=f"xT{ec}")
                  for ec in range(4)]
            wT = {
                w: [pp.tile([128, CH], F16, tag=f"{w}T{ec}", name=f"{w}T{ec}")
                    for ec in range(4)]
                for w in ("wq", "wk", "wv")
            }
            # ErT replicated into both partition halves so each head of a
            # row-tiled pair finds it at its own base partition.
            erT = pp.tile([128, s], F16, tag="erT")
            qT = [pp.tile([128, s], F16, tag=f"qT{oc}", name=f"qT{oc}")
                  for oc in range(NPAIR)]
            kT = [pp.tile([128, s], F16, tag=f"kT{oc}", name=f"kT{oc}")
                  for oc in range(NPAIR)]
            # v with a ones column appended per head: [128, HLOC*(HD+1)] fp16
            vaug = [pp.tile([128, HLOC * (HD + 1)], F16, tag=f"va{sb}",
                            name=f"va{sb}") for sb in range(SB)]
            outp = [pp.tile([128, CH], F32, tag=f"op{sb}", name=f"op{sb}")
                    for sb in range(SB)]

            # ===== setup: load + transpose + projections, interleaved =====
            # Weights and Er first (small DMAs), then per x-block:
            # transpose, v-projection for that block, and the q/k
            # projections for each 512-col chunk as soon as its 4 blocks
            # are resident -- instead of three serialized sweeps.
            with (
                tc.tile_pool(name="ld", bufs=4) as ld_pool,
                tc.tile_pool(name="ps_set", bufs=3, space="PSUM") as ps_set,
                tc.tile_pool(name="ps_pj", bufs=2, space="PSUM") as ps_pj,
            ):
                # Warm-up: absorb the Pool (identity-creation) dependency
                # into PE's observed clock.
                warm = ps_set.tile([128, 128], F32, tag="pset")
                nc.tensor.matmul(
                    warm[:], ident_f32[:], ident_f32[:], start=True, stop=True)

                # weights
                for w_name, w_d in (("wq", wq_d), ("wk", wk_d), ("wv", wv_d)):
                    for pc in range(CH // 128):
                        wt = ld_pool.tile([128, E], F32, tag="wld")
                        nc.sync.dma_start(
                            wt[:], w_d[pc * 128:(pc + 1) * 128, :])
                        for ec in range(4):
                            pst = ps_set.tile([128, 128], F32, tag="pset")
                            nc.tensor.transpose(
                                pst[:], wt[:, ec * 128:(ec + 1) * 128],
                                ident_f32[:])
                            if (pc + ec) % 2 == 0:
                                nc.scalar.copy(
                                    wT[w_name][ec][:, pc * 128:(pc + 1) * 128],
                                    pst[:])
                            else:
                                nc.vector.tensor_copy(
                                    wT[w_name][ec][:, pc * 128:(pc + 1) * 128],
                                    pst[:])
                # Er: transpose into both partition halves, one DVE copy
                for sb in range(SB):
                    et = ld_pool.tile([128, HD], F32, tag="eld")
                    nc.sync.dma_start(et[:], er_d[sb * 128:(sb + 1) * 128, :])
                    pst = ps_set.tile([128, 128], F32, tag="psete")
                    nc.tensor.transpose(pst[0:64, :], et[:], ident_f32[:])
                    nc.tensor.matmul(
                        pst[64:128, :], et[:], ident_f32[:],
                        start=True, stop=True)
                    nc.vector.tensor_copy(
                        erT[:, sb * 128:(sb + 1) * 128], pst[:])
                # x blocks: transpose + v-proj per block, q/k per chunk
                for sb in range(SB):
                    xt = ld_pool.tile([128, E], F32, tag="xld")
                    nc.sync.dma_start(xt[:], x_d[sb * 128:(sb + 1) * 128, :])
                    for ec in range(4):
                        pst = ps_set.tile([128, 128], F32, tag="pset")
                        nc.tensor.transpose(
                            pst[:], xt[:, ec * 128:(ec + 1) * 128],
                            ident_f32[:])
                        if (sb + ec) % 2 == 0:
                            nc.vector.tensor_copy(
                                xT[ec][:, sb * 128:(sb + 1) * 128], pst[:])
                        else:
                            nc.scalar.copy(
                                xT[ec][:, sb * 128:(sb + 1) * 128], pst[:])
                    # v natural + ones column for this block
                    ps = ps_pj.tile([128, CH], F32, tag="pj")
                    for ec in range(4):
                        nc.tensor.matmul(
                            ps[:],
                            xT[ec][:, sb * 128:(sb + 1) * 128],
                            wT["wv"][ec][:],
                            start=(ec == 0), stop=(ec == 3))
                    va = vaug[sb][:].rearrange("p (h d) -> p h d", h=HLOC)
                    nc.vector.tensor_copy(
                        va[:, :, 0:HD],
                        ps[:].rearrange("p (h d) -> p h d", h=HLOC))
                    nc.vector.memset(va[:, :, HD:HD + 1], 1.0)
                    # q/k projections for chunk sc once blocks 4sc..4sc+3
                    # are transposed
                    if sb % 4 == 3:
                        sc = sb // 4
                        for dst, w_name in ((qT, "wq"), (kT, "wk")):
                            for oc in range(NPAIR):
                                ps = ps_pj.tile([128, 512], F32, tag="pj")
                                for ec in range(4):
                                    nc.tensor.matmul(
                                        ps[:],
                                        wT[w_name][ec][:, oc * 128:
                                                       (oc + 1) * 128],
                                        xT[ec][:, sc * 512:(sc + 1) * 512],
                                        start=(ec == 0), stop=(ec == 3))
                                if (oc + sc) % 2 == 0:
                                    nc.scalar.copy(
                                        dst[oc][:, sc * 512:(sc + 1) * 512],
                                        ps[:])
                                else:
                                    nc.vector.tensor_copy(
                                        dst[oc][:, sc * 512:(sc + 1) * 512],
                                        ps[:])

            # ================= projections =================
            with (
                tc.tile_pool(name="wrk", bufs=4) as wrk,
                tc.tile_pool(name="wrk2", bufs=3) as wrk2,
                tc.tile_pool(name="wrk3", bufs=5) as wrk3,
                tc.tile_pool(name="pex", bufs=3) as pex,
                tc.tile_pool(name="srl", bufs=3) as srl,
                tc.tile_pool(name="small", bufs=8) as small,
            ):
                # ---------------- per-head-pair attention ----------------
                from contextlib import ExitStack
                att_stk = ExitStack()
                ps_half = att_stk.enter_context(tc.tile_pool(
                    name="ps_half", bufs=2, space="PSUM"))
                ps_tr = att_stk.enter_context(tc.tile_pool(
                    name="ps_tr", bufs=2, space="PSUM"))
                ps_av = att_stk.enter_context(tc.tile_pool(
                    name="ps_av", bufs=2, space="PSUM"))

                # evac engine alternation: ~1 in 5 goes to ACT (the rest
                # to DVE) so both engines' totals balance given ACT also
                # owns the exps.
                evac_ctr = [0]

                def evac_copy(dst, src):
                    evac_ctr[0] += 1
                    if evac_ctr[0] % 5 == 0:
                        nc.scalar.copy(dst, src)
                    else:
                        nc.vector.tensor_copy(dst, src)

                def phase_a_prep(p, sb):
                    """Allocate padded-P fp8 tiles for heads (2p, 2p+1)."""
                    pexpA = pex.tile([128, s + 1], FP8, tag="pexA",
                                     name="pexpA")
                    pexpB = pex.tile([128, s + 1], FP8, tag="pexB",
                                     name="pexpB")
                    nc.vector.memset(pexpA[:, 0:1], 0.0)
                    nc.vector.memset(pexpB[:, 0:1], 0.0)
                    return (pexpA, pexpB)

                def phase_a_chunk(p, sb, pexps, c):
                    """QEr chunk c for the row-tiled head pair + evac."""
                    qcols = (sb * 128, (sb + 1) * 128)
                    c0 = c * CW
                    psA = ps_half.tile([128, CW], F32, tag="ph", name="psA")
                    psB = ps_half.tile([128, CW], F32, tag="ph", name="psB")
                    for w in range(CW // 512):
                        w0 = c0 + w * 512
                        nc.tensor.matmul(
                            psA[:, w * 512:(w + 1) * 512],
                            qT[p][0:64, qcols[0]:qcols[1]],
                            erT[0:64, w0:w0 + 512],
                            start=True, stop=True)
                        nc.tensor.matmul(
                            psB[:, w * 512:(w + 1) * 512],
                            qT[p][64:128, qcols[0]:qcols[1]],
                            erT[64:128, w0:w0 + 512],
                            start=True, stop=True)
                    evac_copy(pexps[0][:, 1 + c0:1 + c0 + CW], psA[:])
                    evac_copy(pexps[1][:, 1 + c0:1 + c0 + CW], psB[:])

                def phase_a_finish(p, sb, pexps):
                    for h, pexp in ((2 * p, pexps[0]), (2 * p + 1, pexps[1])):
                        nc.sync.dma_start(
                            p_d[h, sb * 128 * (s + 1):
                                (sb * 128 + 128) * (s + 1)]
                            .rearrange("(r c) -> r c", c=s + 1),
                            pexp[:])

                def phase_a_pair(p, sb):
                    pexps = phase_a_prep(p, sb)
                    for c in range(NCH):
                        phase_a_chunk(p, sb, pexps, c)
                    phase_a_finish(p, sb, pexps)

                def phase_b_front(p, sb):
                    """QK matmuls + srel add for the pair -> fp16 scores.

                    With DMA_ACCUM_SREL the QK psum is cast-evacuated to
                    sc_t and the skewed fp8 srel is added in the SDMA
                    datapath (SWDGE accumulate DMA, no engine cost);
                    otherwise DVE tensor_adds an SBUF fp8 srel tile."""
                    hA, hB = 2 * p, 2 * p + 1
                    qcols = (sb * 128, (sb + 1) * 128)
                    base = (sb * 128 + 1) * s
                    srels = []
                    if not DMA_ACCUM_SREL:
                        for h in (hA, hB):
                            srel = srl.tile([128, s], FP8, tag=f"sr{h % 2}",
                                            name="srel")
                            nc.sync.dma_start(
                                srel[:],
                                p_d[h, base:base + 128 * s]
                                .rearrange("(r c) -> r c", c=s))
                            srels.append(srel)
                    sc_ts = []
                    for hi in range(2):
                        sc_t = wrk2.tile([128, s], F16, tag=f"sc{hi}",
                                         name="sc_t")
                        sc_ts.append(sc_t)
                    for c in range(NCH):
                        c0 = c * CW
                        psA = ps_half.tile([128, CW], F32, tag="ph",
                                           name="psQA")
                        psB = ps_half.tile([128, CW], F32, tag="ph",
                                           name="psQB")
                        for w in range(CW // 512):
                            w0 = c0 + w * 512
                            nc.tensor.matmul(
                                psA[:, w * 512:(w + 1) * 512],
                                qT[p][0:64, qcols[0]:qcols[1]],
                                kT[p][0:64, w0:w0 + 512],
                                start=True, stop=True)
                            nc.tensor.matmul(
                                psB[:, w * 512:(w + 1) * 512],
                                qT[p][64:128, qcols[0]:qcols[1]],
                                kT[p][64:128, w0:w0 + 512],
                                start=True, stop=True)
                        if DMA_ACCUM_SREL:
                            for hi, ps in ((0, psA), (1, psB)):
                                evac_copy(sc_ts[hi][:, c0:c0 + CW], ps[:])
                                nc.gpsimd.dma_start(
                                    sc_ts[hi][:, c0:c0 + CW],
                                    p_d[(hA, hB)[hi], base:base + 128 * s]
                                    .rearrange("(r c) -> r c", c=s)
                                    [:, c0:c0 + CW],
                                    accum_op=mybir.AluOpType.add)
                        else:
                            nc.vector.tensor_add(
                                sc_ts[0][:, c0:c0 + CW], psA[:],
                                srels[0][:, c0:c0 + CW])
                            nc.vector.tensor_add(
                                sc_ts[1][:, c0:c0 + CW], psB[:],
                                srels[1][:, c0:c0 + CW])
                    return sc_ts

                def phase_b_texp(p, sb, sc_ts, fillers):
                    """transpose+exp for both heads.  `fillers` are
                    emitted between transpose blocks: transpose-mode
                    matmuls do not register as PE activity for the HAM
                    clock gate, so real matmuls (next pair's QEr, the
                    previous block's AV) are interleaved to keep the PE
                    clock at 8/8."""
                    tpss = []
                    fi = 0
                    for hi in range(2):
                        for half in range(NHALF):
                            pst = ps_tr.tile([128, CW], F16, tag="tr",
                                             name="pst")
                            for t in range(NTH2):
                                c0 = half * CW + t * 128
                                nc.tensor.transpose(
                                    pst[:, t * 128:(t + 1) * 128],
                                    sc_ts[hi][:, c0:c0 + 128],
                                    ident_f16[:])
                            tps = wrk3.tile([128, CW], F16, tag=f"tps{hi}",
                                            name="tps")
                            nc.scalar.activation(
                                tps[:], pst[:],
                                mybir.ActivationFunctionType.Exp, scale=SCALE)
                            tpss.append(tps)
                            if fi < len(fillers):
                                fillers[fi]()
                                fi += 1
                    for f in fillers[fi:]:
                        f()
                    return tpss

                def phase_b_av(p, sb, tpss):
                    """AV matmuls + normalize; runs one block behind
                    texp so the exps have long since drained and the AV
                    matmuls never stall the PE queue."""
                    for hi, h in enumerate((2 * p, 2 * p + 1)):
                        pc_av = ps_av.tile([128, HD + 1], F32, tag="av",
                                           name="pc_av")
                        for half in range(NHALF):
                            tps = tpss[hi * NHALF + half]
                            for t in range(NTH2):
                                ci = half * NTH2 + t
                                nc.tensor.matmul(
                                    pc_av[:],
                                    tps[:, t * 128:(t + 1) * 128],
                                    vaug[ci][:, (h % HLOC) * (HD + 1):
                                             (h % HLOC + 1) * (HD + 1)],
                                    start=(ci == 0), stop=(ci == NTH - 1))
                        rinv = small.tile([128, 1], F32, tag="rinv",
                                          name="rinv")
                        nc.vector.reciprocal(rinv[:], pc_av[:, HD:HD + 1])
                        nc.vector.tensor_scalar_mul(
                            outp[sb][:, (h % HLOC) * HD:(h % HLOC + 1) * HD],
                            pc_av[:, 0:HD], rinv[:])

                def ln_stats_block(sb):
                    s1 = small.tile([128, 1], F32, tag="s1", name="s1")
                    nc.vector.reduce_sum(
                        s1[:], outp[sb][:], axis=mybir.AxisListType.X)
                    sq = small.tile([128, 1], F32, tag="sq", name="sq")
                    scr = wrk.tile([128, CH], F32, tag="scr", name="scr")
                    nc.scalar.activation(
                        scr[:], outp[sb][:],
                        mybir.ActivationFunctionType.Square, accum_out=sq[:])
                    nc.sync.dma_start(
                        cc_in[sb * 128:(sb + 1) * 128, 0:1], s1[:])
                    nc.sync.dma_start(
                        cc_in[sb * 128:(sb + 1) * 128, 1:2], sq[:])

                # software pipeline: srel(p, sb) only reads pexp blocks
                # sb and sb+1, so phase A runs a rolling LAG blocks
                # ahead of phase B.  Per iteration the emission order is
                #   [QK matmuls] [T+exp blocks with interleaved fillers]
                # where the fillers are the next phase-A QEr chunks and
                # the PREVIOUS iteration's AV matmuls -- real matmuls
                # between the HAM-invisible transpose stretches, and the
                # AVs trail one iteration so their exps are long done.
                LAG = min(3, SB)
                for sb in range(LAG):
                    phase_a_pair(0, sb)
                carry = None  # (pr, sb, tpss) awaiting AV
                for pr in range(NPAIR):
                    for sb in range(SB):
                        sc_ts = phase_b_front(pr, sb)
                        na, npr = sb + LAG, pr
                        if na >= SB:
                            na -= SB
                            npr += 1
                        fillers = []
                        if npr < NPAIR:
                            pexps = phase_a_prep(npr, na)
                            for c in range(NCH):
                                fillers.append(
                                    lambda npr=npr, na=na, pexps=pexps, c=c:
                                    phase_a_chunk(npr, na, pexps, c))
                        if carry is not None:
                            cpr, csb, ctpss = carry
                            fillers.append(
                                lambda cpr=cpr, csb=csb, ctpss=ctpss:
                                phase_b_av(cpr, csb, ctpss))
                            if cpr + 1 == NPAIR:
                                fillers.append(
                                    lambda csb=csb: ln_stats_block(csb))
                        tpss = phase_b_texp(p=pr, sb=sb, sc_ts=sc_ts,
                                            fillers=fillers)
                        if npr < NPAIR:
                            phase_a_finish(npr, na, pexps)
                        carry = (pr, sb, tpss)
                # drain the last AV
                cpr, csb, ctpss = carry
                phase_b_av(cpr, csb, ctpss)
                ln_stats_block(csb)

                att_stk.close()
                # ================= LayerNorm =================
                nc.gpsimd.collective_compute(
                    "AllReduce", mybir.AluOpType.add,
                    replica_groups=pairs,
                    ins=[cc_in[:].opt()], outs=[cc_out[:].opt()])
                for sb in range(SB):
                    st = small.tile([128, 2], F32, tag="st")
                    nc.sync.dma_start(st[:], cc_out[sb * 128:(sb + 1) * 128, :])
                    mean = small.tile([128, 1], F32, tag="mean")
                    nc.vector.tensor_scalar_mul(mean[:], st[:, 0:1], 1.0 / E)
                    ex2 = small.tile([128, 1], F32, tag="ex2")
                    nc.vector.tensor_scalar_mul(ex2[:], st[:, 1:2], 1.0 / E)
                    msq = small.tile([128, 1], F32, tag="msq")
                    nc.vector.tensor_mul(msq[:], mean[:], mean[:])
                    var = small.tile([128, 1], F32, tag="var")
                    nc.vector.tensor_sub(var[:], ex2[:], msq[:])
                    std = small.tile([128, 1], F32, tag="std")
                    nc.scalar.activation(
                        std[:], var[:],
                        mybir.ActivationFunctionType.Sqrt, bias=eps_t[:])
                    rstd = small.tile([128, 1], F32, tag="rstd")
                    nc.vector.reciprocal(rstd[:], std[:])
                    tmp = wrk.tile([128, CH], F32, tag="tmp")
                    nc.vector.tensor_scalar(
                        tmp[:], outp[sb][:], mean[:], rstd[:],
                        op0=mybir.AluOpType.subtract,
                        op1=mybir.AluOpType.mult)
                    y1 = wrk2.tile([128, CH], F32, tag="y1")
                    nc.vector.tensor_mul(y1[:], tmp[:], gamma_bc[:])
                    y2 = wrk3.tile([128, CH], F32, tag="y2")
                    nc.vector.tensor_add(y2[:], y1[:], beta_bc[:])
                    nc.sync.dma_start(out_d[sb * 128:(sb + 1) * 128, :], y2[:])

    if legalize:
        _legalize_waits(nc)
    return nc


def _legalize_waits(nc):
    """walrus's codegen accepts at most one sync wait on most instruction
    structs; hoist extra waits onto NoOps inserted just before, on the
    same engine queue (program order preserves the semantics)."""
    n = 0
    keep = set()
    for bb in nc.main_func.blocks:
        out = []
        for inst in bb.instructions:
            si = inst.sync_info
            if (inst.opcode not in keep and si is not None
                    and si.on_wait and len(si.on_wait) > 1):
                for w in si.on_wait[:-1]:
                    nop = mybir.InstNoOp(
                        name=f"I-mmw{n}", ins=[], outs=[])
                    n += 1
                    nop.engine = inst.engine
                    nop.sync_info = mybir.SyncInfo(
                        on_wait=[w], on_update=[])
                    out.append(nop)
                si.on_wait = [si.on_wait[-1]]
            out.append(inst)
        bb.instructions = out
    return nc


_NC_CACHE = {}


def _get_nc(s=S, n_cores=N_CORES):
    key = (s, n_cores)
    if key not in _NC_CACHE:
        _NC_CACHE[key] = build_nc(s, n_cores)
    return _NC_CACHE[key]


def make_in_maps(x, Wq, Wk, Wv, Er, gamma, beta, n_cores=N_CORES):
    in_maps = []
    for c in range(n_cores):
        b, hg = c // 2, c % 2
        sl = slice(hg * CH, (hg + 1) * CH)
        in_maps.append({
            "x": np.ascontiguousarray(x[b], dtype=np.float32),
            "wq": np.ascontiguousarray(Wq[sl], dtype=np.float32),
            "wk": np.ascontiguousarray(Wk[sl], dtype=np.float32),
            "wv": np.ascontiguousarray(Wv[sl], dtype=np.float32),
            "er": np.ascontiguousarray(Er, dtype=np.float32),
            "gamma": np.ascontiguousarray(gamma[sl], dtype=np.float32)[None, :],
            "beta": np.ascontiguousarray(beta[sl], dtype=np.float32)[None, :],
        })
    return in_maps


def assemble(results, n_cores=N_CORES, s=S):
    y = np.empty((n_cores // 2, s, E), np.float32)
    for c in range(n_cores):
        y[c // 2, :, (c % 2) * CH:(c % 2 + 1) * CH] = results[c]["out"]
    return y


def kernel(**inputs):
    from concourse.bass_utils import run_bass_kernel_spmd
    nc = _get_nc()
    in_maps = make_in_maps(
        inputs["x"], inputs["Wq"], inputs["Wk"], inputs["Wv"],
        inputs["Er"], inputs["gamma"], inputs["beta"])
    res = run_bass_kernel_spmd(nc, in_maps, list(range(N_CORES)))
    return assemble(res.results)


# revision 32
# speedup vs baseline: 1.0965x; 1.0965x over previous
"""Relative-position attention (Music-Transformer style skew) + LayerNorm,
distributed over 8 TRN2 NeuronCores.

Sharding: data-parallel over batch (B=4) x tensor-parallel over head-halves
(H=8 -> 2 groups of 4). Core c handles batch b=c//2, heads [4*(c%2), 4*(c%2)+4),
producing output channels [256*(c%2), +256) of y[b]. The final LayerNorm needs
full-E stats, exchanged via a tiny pairwise AllReduce of (sum, sumsq).

Skew trick: Srel[i,j] = F[(i+1)*S + j] where F is the row-major flat view of
the padded matrix P[i, 0]=0, P[i, 1+l]=QEr[i, l] (P is [S, S+1]). We bounce P
through DRAM in fp8e4m3; the skewed read back is a plain strided DMA.

Perf notes vs the old fp32r version:
 - All PE-streamed data is fp16 (same PE rate as bf16, ~4x faster than
   fp32r at N=512, FWL on weight loads). PSUM accumulation stays fp32.
 - Heads are processed in row-tiled pairs: head 2p lives in PE rows 0-63,
   head 2p+1 in rows 64-127 (qT/kT/erT halves), so the K=64 QEr/QK matmuls
   of the two heads execute concurrently in the array.
 - The QEr bounce is fp8 (2x less HBM traffic than bf16); measured numpy
   end-to-end error ~6e-3 vs the 2e-2 budget.
 - Elementwise work (PSUM evacuations, score adds) is split between ACT
   and DVE to balance engine busy time; exp runs on ACT from a 2-bank
   PSUM tile in one [128, S] activation per head-block.
"""

import numpy as np

import concourse.bass as bass
import concourse.mybir as mybir
from concourse import masks
from concourse.tile import TileContext

F32 = mybir.dt.float32
F16 = mybir.dt.float16
FP8 = mybir.dt.float8e4

B, S, E, H = 4, 2048, 512, 8
HD = E // H          # 64
HLOC = 4             # heads per core
CH = HLOC * HD       # 256 output channels per core
SCALE = float(E) ** -0.5
EPS = 1e-5
N_CORES = 8
# Srel add strategy: True = SWDGE DMA-accumulate (fp8 DRAM read casts and
# adds into the fp16 score tile in the SDMA datapath, freeing DVE); False
# = DVE tensor_tensor add of (psum, fp8 srel tile).
DMA_ACCUM_SREL = False


def build_nc(s=S, n_cores=N_CORES, debug=False, legalize=True):
    """Build the per-core Bass graph (SPMD: same graph on all cores)."""
    nc = bass.Bass(target_bir_lowering=False, debug=debug)

    SB = s // 128        # number of 128-row blocks
    KC = s // 512        # number of 512-col chunks
    NTH = s // 128       # transpose blocks per scores row-block
    NPAIR = HLOC // 2    # head pairs per core
    # evac/add chunking: pieces of <=1024 cols
    CW = min(1024, s)
    NCH = s // CW
    # transpose/exp half-granularity: [128, CW] fp16 = one PSUM bank
    NHALF = s // CW
    NTH2 = CW // 128     # transposes per half

    x_d = nc.declare_dram_parameter("x", [s, E], F32, isOutput=False)
    wq_d = nc.declare_dram_parameter("wq", [CH, E], F32, isOutput=False)
    wk_d = nc.declare_dram_parameter("wk", [CH, E], F32, isOutput=False)
    wv_d = nc.declare_dram_parameter("wv", [CH, E], F32, isOutput=False)
    er_d = nc.declare_dram_parameter("er", [s, HD], F32, isOutput=False)
    gamma_d = nc.declare_dram_parameter("gamma", [1, CH], F32, isOutput=False)
    beta_d = nc.declare_dram_parameter("beta", [1, CH], F32, isOutput=False)
    out_d = nc.declare_dram_parameter("out", [s, CH], F32, isOutput=True)

    # Padded-QEr bounce buffers, one per head, flat [S*(S+1)] fp8.
    p_d = nc.dram_tensor("pbuf", [HLOC, s * (s + 1)], FP8)
    cc_in = nc.dram_tensor("cc_in", [s, 2], F32)
    cc_out = nc.dram_tensor("cc_out", [s, 2], F32)

    pairs = [[2 * i, 2 * i + 1] for i in range(n_cores // 2)]

    with TileContext(nc) as tc:
        with (
            tc.tile_pool(name="const", bufs=1) as const_pool,
            tc.tile_pool(name="persist", bufs=1) as pp,
        ):
            ident_f32 = const_pool.tile([128, 128], F32)
            ident_f16 = const_pool.tile([128, 128], F16)
            masks.make_identity(nc, ident_f32[:])
            masks.make_identity(nc, ident_f16[:])
            gamma_bc = const_pool.tile([128, CH], F32)
            beta_bc = const_pool.tile([128, CH], F32)
            eps_t = const_pool.tile([128, 1], F32)
            nc.gpsimd.memset(eps_t[:], EPS)
            nc.sync.dma_start(gamma_bc[:], gamma_d[:].broadcast_to((128, CH)))
            nc.sync.dma_start(beta_bc[:], beta_d[:].broadcast_to((128, CH)))

            # ---- persistent SBUF tensors (fp16 for PE streaming) ----
            xT = [pp.tile([128, s], F16, tag=f"xT{ec}", name=f"xT{ec}")
                  for ec in range(4)]
            wT = {
                w: [pp.tile([128, CH], F16, tag=f"{w}T{ec}", name=f"{w}T{ec}")
                    for ec in range(4)]
                for w in ("wq", "wk", "wv")
            }
            # ErT replicated into both partition halves so each head of a
            # row-tiled pair finds it at its own base partition.
            erT = pp.tile([128, s], F16, tag="erT")
            qT = [pp.tile([128, s], F16, tag=f"qT{oc}", name=f"qT{oc}")
                  for oc in range(NPAIR)]
            kT = [pp.tile([128, s], F16, tag=f"kT{oc}", name=f"kT{oc}")
                  for oc in range(NPAIR)]
            # v with a ones column appended per head: [128, HLOC*(HD+1)] fp16
            vaug = [pp.tile([128, HLOC * (HD + 1)], F16, tag=f"va{sb}",
                            name=f"va{sb}") for sb in range(SB)]
            outp = [pp.tile([128, CH], F32, tag=f"op{sb}", name=f"op{sb}")
                    for sb in range(SB)]

            # ================= setup: load + transpose =================
            with (
                tc.tile_pool(name="ld", bufs=4) as ld_pool,
                tc.tile_pool(name="ps_set", bufs=4, space="PSUM") as ps_set,
            ):
                # Warm-up: absorb the Pool (identity-creation) dependency
                # into PE's observed clock.
                warm = ps_set.tile([128, 128], F32, tag="pset")
                nc.tensor.matmul(
                    warm[:], ident_f32[:], ident_f32[:], start=True, stop=True)

                # xT[ec][:, i*128:(i+1)*128] = x[i-block, ec-block].T
                for sb in range(SB):
                    xt = ld_pool.tile([128, E], F32, tag="xld")
                    nc.sync.dma_start(xt[:], x_d[sb * 128:(sb + 1) * 128, :])
                    for ec in range(4):
                        pst = ps_set.tile([128, 128], F32, tag="pset")
                        nc.tensor.transpose(
                            pst[:], xt[:, ec * 128:(ec + 1) * 128],
                            ident_f32[:])
                        if (sb + ec) % 2 == 0:
                            nc.vector.tensor_copy(
                                xT[ec][:, sb * 128:(sb + 1) * 128], pst[:])
                        else:
                            nc.scalar.copy(
                                xT[ec][:, sb * 128:(sb + 1) * 128], pst[:])
                # weights
                for w_name, w_d in (("wq", wq_d), ("wk", wk_d), ("wv", wv_d)):
                    for pc in range(CH // 128):
                        wt = ld_pool.tile([128, E], F32, tag="wld")
                        nc.sync.dma_start(
                            wt[:], w_d[pc * 128:(pc + 1) * 128, :])
                        for ec in range(4):
                            pst = ps_set.tile([128, 128], F32, tag="pset")
                            nc.tensor.transpose(
                                pst[:], wt[:, ec * 128:(ec + 1) * 128],
                                ident_f32[:])
                            if (pc + ec) % 2 == 0:
                                nc.scalar.copy(
                                    wT[w_name][ec][:, pc * 128:(pc + 1) * 128],
                                    pst[:])
                            else:
                                nc.vector.tensor_copy(
                                    wT[w_name][ec][:, pc * 128:(pc + 1) * 128],
                                    pst[:])
                # Er: transpose into both partition halves, one DVE copy
                for sb in range(SB):
                    et = ld_pool.tile([128, HD], F32, tag="eld")
                    nc.sync.dma_start(et[:], er_d[sb * 128:(sb + 1) * 128, :])
                    pst = ps_set.tile([128, 128], F32, tag="psete")
                    nc.tensor.transpose(pst[0:64, :], et[:], ident_f32[:])
                    nc.tensor.matmul(
                        pst[64:128, :], et[:], ident_f32[:],
                        start=True, stop=True)
                    nc.vector.tensor_copy(
                        erT[:, sb * 128:(sb + 1) * 128], pst[:])

            # ================= projections =================
            with (
                tc.tile_pool(name="wrk", bufs=4) as wrk,
                tc.tile_pool(name="wrk2", bufs=3) as wrk2,
                tc.tile_pool(name="wrk3", bufs=5) as wrk3,
                tc.tile_pool(name="pex", bufs=3) as pex,
                tc.tile_pool(name="srl", bufs=3) as srl,
                tc.tile_pool(name="small", bufs=8) as small,
            ):
                with tc.tile_pool(
                        name="ps_pj", bufs=4, space="PSUM") as ps_pj:
                    # qT / kT: [oc*128+p, t] = sum_e W[oc*128+p, e] x[t, e]
                    for dst, w_name in ((qT, "wq"), (kT, "wk")):
                        for oc in range(NPAIR):
                            for sc in range(KC):
                                ps = ps_pj.tile([128, 512], F32, tag="pj")
                                for ec in range(4):
                                    nc.tensor.matmul(
                                        ps[:],
                                        wT[w_name][ec][:, oc * 128:
                                                       (oc + 1) * 128],
                                        xT[ec][:, sc * 512:(sc + 1) * 512],
                                        start=(ec == 0), stop=(ec == 3))
                                if (oc + sc) % 2 == 0:
                                    nc.scalar.copy(
                                        dst[oc][:, sc * 512:(sc + 1) * 512],
                                        ps[:])
                                else:
                                    nc.vector.tensor_copy(
                                        dst[oc][:, sc * 512:(sc + 1) * 512],
                                        ps[:])
                    # v natural + ones column, fp16; one strided copy per
                    # block moves all 4 heads, one strided memset the ones.
                    for sb in range(SB):
                        ps = ps_pj.tile([128, CH], F32, tag="pj")
                        for ec in range(4):
                            nc.tensor.matmul(
                                ps[:],
                                xT[ec][:, sb * 128:(sb + 1) * 128],
                                wT["wv"][ec][:],
                                start=(ec == 0), stop=(ec == 3))
                        va = vaug[sb][:].rearrange(
                            "p (h d) -> p h d", h=HLOC)
                        nc.vector.tensor_copy(
                            va[:, :, 0:HD],
                            ps[:].rearrange("p (h d) -> p h d", h=HLOC))
                        nc.vector.memset(va[:, :, HD:HD + 1], 1.0)

                # ---------------- per-head-pair attention ----------------
                from contextlib import ExitStack
                att_stk = ExitStack()
                ps_half = att_stk.enter_context(tc.tile_pool(
                    name="ps_half", bufs=2, space="PSUM"))
                ps_tr = att_stk.enter_context(tc.tile_pool(
                    name="ps_tr", bufs=2, space="PSUM"))
                ps_av = att_stk.enter_context(tc.tile_pool(
                    name="ps_av", bufs=2, space="PSUM"))

                # evac engine alternation: ~1 in 5 goes to ACT (the rest
                # to DVE) so both engines' totals balance given ACT also
                # owns the exps.
                evac_ctr = [0]

                def evac_copy(dst, src):
                    evac_ctr[0] += 1
                    if evac_ctr[0] % 5 == 0:
                        nc.scalar.copy(dst, src)
                    else:
                        nc.vector.tensor_copy(dst, src)

                def phase_a_prep(p, sb):
                    """Allocate padded-P fp8 tiles for heads (2p, 2p+1)."""
                    pexpA = pex.tile([128, s + 1], FP8, tag="pexA",
                                     name="pexpA")
                    pexpB = pex.tile([128, s + 1], FP8, tag="pexB",
                                     name="pexpB")
                    nc.vector.memset(pexpA[:, 0:1], 0.0)
                    nc.vector.memset(pexpB[:, 0:1], 0.0)
                    return (pexpA, pexpB)

                def phase_a_chunk(p, sb, pexps, c):
                    """QEr chunk c for the row-tiled head pair + evac."""
                    qcols = (sb * 128, (sb + 1) * 128)
                    c0 = c * CW
                    psA = ps_half.tile([128, CW], F32, tag="ph", name="psA")
                    psB = ps_half.tile([128, CW], F32, tag="ph", name="psB")
                    for w in range(CW // 512):
                        w0 = c0 + w * 512
                        nc.tensor.matmul(
                            psA[:, w * 512:(w + 1) * 512],
                            qT[p][0:64, qcols[0]:qcols[1]],
                            erT[0:64, w0:w0 + 512],
                            start=True, stop=True)
                        nc.tensor.matmul(
                            psB[:, w * 512:(w + 1) * 512],
                            qT[p][64:128, qcols[0]:qcols[1]],
                            erT[64:128, w0:w0 + 512],
                            start=True, stop=True)
                    evac_copy(pexps[0][:, 1 + c0:1 + c0 + CW], psA[:])
                    evac_copy(pexps[1][:, 1 + c0:1 + c0 + CW], psB[:])

                def phase_a_finish(p, sb, pexps):
                    for h, pexp in ((2 * p, pexps[0]), (2 * p + 1, pexps[1])):
                        nc.sync.dma_start(
                            p_d[h, sb * 128 * (s + 1):
                                (sb * 128 + 128) * (s + 1)]
                            .rearrange("(r c) -> r c", c=s + 1),
                            pexp[:])

                def phase_a_pair(p, sb):
                    pexps = phase_a_prep(p, sb)
                    for c in range(NCH):
                        phase_a_chunk(p, sb, pexps, c)
                    phase_a_finish(p, sb, pexps)

                def phase_b_front(p, sb):
                    """QK matmuls + srel add for the pair -> fp16 scores.

                    With DMA_ACCUM_SREL the QK psum is cast-evacuated to
                    sc_t and the skewed fp8 srel is added in the SDMA
                    datapath (SWDGE accumulate DMA, no engine cost);
                    otherwise DVE tensor_adds an SBUF fp8 srel tile."""
                    hA, hB = 2 * p, 2 * p + 1
                    qcols = (sb * 128, (sb + 1) * 128)
                    base = (sb * 128 + 1) * s
                    srels = []
                    if not DMA_ACCUM_SREL:
                        for h in (hA, hB):
                            srel = srl.tile([128, s], FP8, tag=f"sr{h % 2}",
                                            name="srel")
                            nc.sync.dma_start(
                                srel[:],
                                p_d[h, base:base + 128 * s]
                                .rearrange("(r c) -> r c", c=s))
                            srels.append(srel)
                    sc_ts = []
                    for hi in range(2):
                        sc_t = wrk2.tile([128, s], F16, tag=f"sc{hi}",
                                         name="sc_t")
                        sc_ts.append(sc_t)
                    for c in range(NCH):
                        c0 = c * CW
                        psA = ps_half.tile([128, CW], F32, tag="ph",
                                           name="psQA")
                        psB = ps_half.tile([128, CW], F32, tag="ph",
                                           name="psQB")
                        for w in range(CW // 512):
                            w0 = c0 + w * 512
                            nc.tensor.matmul(
                                psA[:, w * 512:(w + 1) * 512],
                                qT[p][0:64, qcols[0]:qcols[1]],
                                kT[p][0:64, w0:w0 + 512],
                                start=True, stop=True)
                            nc.tensor.matmul(
                                psB[:, w * 512:(w + 1) * 512],
                                qT[p][64:128, qcols[0]:qcols[1]],
                                kT[p][64:128, w0:w0 + 512],
                                start=True, stop=True)
                        if DMA_ACCUM_SREL:
                            for hi, ps in ((0, psA), (1, psB)):
                                evac_copy(sc_ts[hi][:, c0:c0 + CW], ps[:])
                                nc.gpsimd.dma_start(
                                    sc_ts[hi][:, c0:c0 + CW],
                                    p_d[(hA, hB)[hi], base:base + 128 * s]
                                    .rearrange("(r c) -> r c", c=s)
                                    [:, c0:c0 + CW],
                                    accum_op=mybir.AluOpType.add)
                        else:
                            nc.vector.tensor_add(
                                sc_ts[0][:, c0:c0 + CW], psA[:],
                                srels[0][:, c0:c0 + CW])
                            nc.vector.tensor_add(
                                sc_ts[1][:, c0:c0 + CW], psB[:],
                                srels[1][:, c0:c0 + CW])
                    return sc_ts

                def phase_b_texp(p, sb, sc_ts, fillers):
                    """transpose+exp for both heads.  `fillers` are
                    emitted between transpose blocks: transpose-mode
                    matmuls do not register as PE activity for the HAM
                    clock gate, so real matmuls (next pair's QEr, the
                    previous block's AV) are interleaved to keep the PE
                    clock at 8/8."""
                    tpss = []
                    fi = 0
                    for hi in range(2):
                        for half in range(NHALF):
                            pst = ps_tr.tile([128, CW], F16, tag="tr",
                                             name="pst")
                            for t in range(NTH2):
                                c0 = half * CW + t * 128
                                nc.tensor.transpose(
                                    pst[:, t * 128:(t + 1) * 128],
                                    sc_ts[hi][:, c0:c0 + 128],
                                    ident_f16[:])
                            tps = wrk3.tile([128, CW], F16, tag=f"tps{hi}",
                                            name="tps")
                            nc.scalar.activation(
                                tps[:], pst[:],
                                mybir.ActivationFunctionType.Exp, scale=SCALE)
                            tpss.append(tps)
                            if fi < len(fillers):
                                fillers[fi]()
                                fi += 1
                    for f in fillers[fi:]:
                        f()
                    return tpss

                def phase_b_av(p, sb, tpss):
                    """AV matmuls + normalize; runs one block behind
                    texp so the exps have long since drained and the AV
                    matmuls never stall the PE queue."""
                    for hi, h in enumerate((2 * p, 2 * p + 1)):
                        pc_av = ps_av.tile([128, HD + 1], F32, tag="av",
                                           name="pc_av")
                        for half in range(NHALF):
                            tps = tpss[hi * NHALF + half]
                            for t in range(NTH2):
                                ci = half * NTH2 + t
                                nc.tensor.matmul(
                                    pc_av[:],
                                    tps[:, t * 128:(t + 1) * 128],
                                    vaug[ci][:, (h % HLOC) * (HD + 1):
                                             (h % HLOC + 1) * (HD + 1)],
                                    start=(ci == 0), stop=(ci == NTH - 1))
                        rinv = small.tile([128, 1], F32, tag="rinv",
                                          name="rinv")
                        nc.vector.reciprocal(rinv[:], pc_av[:, HD:HD + 1])
                        nc.vector.tensor_scalar_mul(
                            outp[sb][:, (h % HLOC) * HD:(h % HLOC + 1) * HD],
                            pc_av[:, 0:HD], rinv[:])

                def ln_stats_block(sb):
                    s1 = small.tile([128, 1], F32, tag="s1", name="s1")
                    nc.vector.reduce_sum(
                        s1[:], outp[sb][:], axis=mybir.AxisListType.X)
                    sq = small.tile([128, 1], F32, tag="sq", name="sq")
                    scr = wrk.tile([128, CH], F32, tag="scr", name="scr")
                    nc.scalar.activation(
                        scr[:], outp[sb][:],
                        mybir.ActivationFunctionType.Square, accum_out=sq[:])
                    nc.sync.dma_start(
                        cc_in[sb * 128:(sb + 1) * 128, 0:1], s1[:])
                    nc.sync.dma_start(
                        cc_in[sb * 128:(sb + 1) * 128, 1:2], sq[:])

                # software pipeline: srel(p, sb) only reads pexp blocks
                # sb and sb+1, so phase A runs a rolling LAG blocks
                # ahead of phase B.  Per iteration the emission order is
                #   [QK matmuls] [T+exp blocks with interleaved fillers]
                # where the fillers are the next phase-A QEr chunks and
                # the PREVIOUS iteration's AV matmuls -- real matmuls
                # between the HAM-invisible transpose stretches, and the
                # AVs trail one iteration so their exps are long done.
                LAG = min(3, SB)
                for sb in range(LAG):
                    phase_a_pair(0, sb)
                carry = None  # (pr, sb, tpss) awaiting AV
                for pr in range(NPAIR):
                    for sb in range(SB):
                        sc_ts = phase_b_front(pr, sb)
                        na, npr = sb + LAG, pr
                        if na >= SB:
                            na -= SB
                            npr += 1
                        fillers = []
                        if npr < NPAIR:
                            pexps = phase_a_prep(npr, na)
                            for c in range(NCH):
                                fillers.append(
                                    lambda npr=npr, na=na, pexps=pexps, c=c:
                                    phase_a_chunk(npr, na, pexps, c))
                        if carry is not None:
                            cpr, csb, ctpss = carry
                            fillers.append(
                                lambda cpr=cpr, csb=csb, ctpss=ctpss:
                                phase_b_av(cpr, csb, ctpss))
                            if cpr + 1 == NPAIR:
                                fillers.append(
                                    lambda csb=csb: ln_stats_block(csb))
                        tpss = phase_b_texp(p=pr, sb=sb, sc_ts=sc_ts,
                                            fillers=fillers)
                        if npr < NPAIR:
                            phase_a_finish(npr, na, pexps)
                        carry = (pr, sb, tpss)
                # drain the last AV
                cpr, csb, ctpss = carry
                phase_b_av(cpr, csb, ctpss)
                ln_stats_block(csb)

                att_stk.close()
                # ================= LayerNorm =================
                nc.gpsimd.collective_compute(
                    "AllReduce", mybir.AluOpType.add,
                    replica_groups=pairs,
                    ins=[cc_in[:].opt()], outs=[cc_out[:].opt()])
                for sb in range(SB):
                    st = small.tile([128, 2], F32, tag="st")
                    nc.sync.dma_start(st[:], cc_out[sb * 128:(sb + 1) * 128, :])
                    mean = small.tile([128, 1], F32, tag="mean")
                    nc.vector.tensor_scalar_mul(mean[:], st[:, 0:1], 1.0 / E)
                    ex2 = small.tile([128, 1], F32, tag="ex2")
                    nc.vector.tensor_scalar_mul(ex2[:], st[:, 1:2], 1.0 / E)
                    msq = small.tile([128, 1], F32, tag="msq")
                    nc.vector.tensor_mul(msq[:], mean[:], mean[:])
                    var = small.tile([128, 1], F32, tag="var")
                    nc.vector.tensor_sub(var[:], ex2[:], msq[:])
                    std = small.tile([128, 1], F32, tag="std")
                    nc.scalar.activation(
                        std[:], var[:],
                        mybir.ActivationFunctionType.Sqrt, bias=eps_t[:])
                    rstd = small.tile([128, 1], F32, tag="rstd")
                    nc.vector.reciprocal(rstd[:], std[:])
                    tmp = wrk.tile([128, CH], F32, tag="tmp")
                    nc.vector.tensor_scalar(
                        tmp[:], outp[sb][:], mean[:], rstd[:],
                        op0=mybir.AluOpType.subtract,
                        op1=mybir.AluOpType.mult)
                    y1 = wrk2.tile([128, CH], F32, tag="y1")
                    nc.vector.tensor_mul(y1[:], tmp[:], gamma_bc[:])
                    y2 = wrk3.tile([128, CH], F32, tag="y2")
                    nc.vector.tensor_add(y2[:], y1[:], beta_bc[:])
                    nc.sync.dma_start(out_d[sb * 128:(sb + 1) * 128, :], y2[:])

    if legalize:
        _legalize_waits(nc)
    return nc


def _legalize_waits(nc):
    """walrus's codegen accepts at most one sync wait on most instruction
    structs; hoist extra waits onto NoOps inserted just before, on the
    same engine queue (program order preserves the semantics)."""
    n = 0
    keep = set()
    for bb in nc.main_func.blocks:
        out = []
        for inst in bb.instructions:
            si = inst.sync_info
            if (inst.opcode not in keep and si is not None
                    and si.on_wait and len(si.on_wait) > 1):
                for w in si.on_wait[:-1]:
                    nop = mybir.InstNoOp(
                        name=f"I-mmw{n}", ins=[], outs=[])
                    n += 1
                    nop.engine = inst.engine
                    nop.sync_info = mybir.SyncInfo(
                        on_wait=[w], on_update=[])
                    out.append(nop)
                si.on_wait = [si.on_wait[-1]]
            out.append(inst)
        bb.instructions = out
    return nc


_NC_CACHE = {}


def _get_nc(s=S, n_cores=N_CORES):
    key = (s, n_cores)
    if key not in _NC_CACHE:
        _NC_CACHE[key] = build_nc(s, n_cores)
    return _NC_CACHE[key]


def make_in_maps(x, Wq, Wk, Wv, Er, gamma, beta, n_cores=N_CORES):
    in_maps = []
    for c in range(n_cores):
        b, hg = c // 2, c % 2
        sl = slice(hg * CH, (hg + 1) * CH)
        in_maps.append({
            "x": np.ascontiguousarray(x[b], dtype=np.float32),
            "wq": np.ascontiguousarray(Wq[sl], dtype=np.float32),
            "wk": np.ascontiguousarray(Wk[sl], dtype=np.float32),
            "wv": np.ascontiguousarray(Wv[sl], dtype=np.float32),
            "er": np.ascontiguousarray(Er, dtype=np.float32),
            "gamma": np.ascontiguousarray(gamma[sl], dtype=np.float32)[None, :],
            "beta": np.ascontiguousarray(beta[sl], dtype=np.float32)[None, :],
        })
    return in_maps


def assemble(results, n_cores=N_CORES, s=S):
    y = np.empty((n_cores // 2, s, E), np.float32)
    for c in range(n_cores):
        y[c // 2, :, (c % 2) * CH:(c % 2 + 1) * CH] = results[c]["out"]
    return y


def kernel(**inputs):
    from concourse.bass_utils import run_bass_kernel_spmd
    nc = _get_nc()
    in_maps = make_in_maps(
        inputs["x"], inputs["Wq"], inputs["Wk"], inputs["Wv"],
        inputs["Er"], inputs["gamma"], inputs["beta"])
    res = run_bass_kernel_spmd(nc, in_maps, list(range(N_CORES)))
    return assemble(res.results)


# revision 35
# speedup vs baseline: 1.1056x; 1.0083x over previous
"""Relative-position attention (Music-Transformer style skew) + LayerNorm,
distributed over 8 TRN2 NeuronCores.

Sharding: data-parallel over batch (B=4) x tensor-parallel over head-halves
(H=8 -> 2 groups of 4). Core c handles batch b=c//2, heads [4*(c%2), 4*(c%2)+4),
producing output channels [256*(c%2), +256) of y[b]. The final LayerNorm needs
full-E stats, exchanged via a tiny pairwise AllReduce of (sum, sumsq).

Skew trick: Srel[i,j] = F[(i+1)*S + j] where F is the row-major flat view of
the padded matrix P[i, 0]=0, P[i, 1+l]=QEr[i, l] (P is [S, S+1]). We bounce P
through DRAM in fp8e4m3; the skewed read back is a plain strided DMA.

Perf notes vs the old fp32r version:
 - All PE-streamed data is fp16 (same PE rate as bf16, ~4x faster than
   fp32r at N=512, FWL on weight loads). PSUM accumulation stays fp32.
 - Heads are processed in row-tiled pairs: head 2p lives in PE rows 0-63,
   head 2p+1 in rows 64-127 (qT/kT/erT halves), so the K=64 QEr/QK matmuls
   of the two heads execute concurrently in the array.
 - The QEr bounce is fp8 (2x less HBM traffic than bf16); measured numpy
   end-to-end error ~6e-3 vs the 2e-2 budget.
 - Elementwise work (PSUM evacuations, score adds) is split between ACT
   and DVE to balance engine busy time; exp runs on ACT from a 2-bank
   PSUM tile in one [128, S] activation per head-block.
"""

import numpy as np

import concourse.bass as bass
import concourse.mybir as mybir
from concourse import masks
from concourse.tile import TileContext

F32 = mybir.dt.float32
F16 = mybir.dt.float16
FP8 = mybir.dt.float8e4

B, S, E, H = 4, 2048, 512, 8
HD = E // H          # 64
HLOC = 4             # heads per core
CH = HLOC * HD       # 256 output channels per core
SCALE = float(E) ** -0.5
EPS = 1e-5
N_CORES = 8
# Srel add strategy: True = SWDGE DMA-accumulate (fp8 DRAM read casts and
# adds into the fp16 score tile in the SDMA datapath, freeing DVE); False
# = DVE tensor_tensor add of (psum, fp8 srel tile).
DMA_ACCUM_SREL = False


def build_nc(s=S, n_cores=N_CORES, debug=False, legalize=True):
    """Build the per-core Bass graph (SPMD: same graph on all cores)."""
    nc = bass.Bass(target_bir_lowering=False, debug=debug)

    SB = s // 128        # number of 128-row blocks
    KC = s // 512        # number of 512-col chunks
    NTH = s // 128       # transpose blocks per scores row-block
    NPAIR = HLOC // 2    # head pairs per core
    # evac/add chunking: pieces of <=1024 cols
    CW = min(1024, s)
    NCH = s // CW
    # transpose/exp half-granularity: [128, CW] fp16 = one PSUM bank
    NHALF = s // CW
    NTH2 = CW // 128     # transposes per half

    x_d = nc.declare_dram_parameter("x", [s, E], F32, isOutput=False)
    wq_d = nc.declare_dram_parameter("wq", [CH, E], F32, isOutput=False)
    wk_d = nc.declare_dram_parameter("wk", [CH, E], F32, isOutput=False)
    wv_d = nc.declare_dram_parameter("wv", [CH, E], F32, isOutput=False)
    er_d = nc.declare_dram_parameter("er", [s, HD], F32, isOutput=False)
    gamma_d = nc.declare_dram_parameter("gamma", [1, CH], F32, isOutput=False)
    beta_d = nc.declare_dram_parameter("beta", [1, CH], F32, isOutput=False)
    out_d = nc.declare_dram_parameter("out", [s, CH], F32, isOutput=True)

    # Padded-QEr bounce buffers, one per head, flat [S*(S+1)] fp8.
    p_d = nc.dram_tensor("pbuf", [HLOC, s * (s + 1)], FP8)
    cc_in = nc.dram_tensor("cc_in", [s, 2], F32)
    cc_out = nc.dram_tensor("cc_out", [s, 2], F32)

    pairs = [[2 * i, 2 * i + 1] for i in range(n_cores // 2)]

    with TileContext(nc) as tc:
        with (
            tc.tile_pool(name="const", bufs=1) as const_pool,
            tc.tile_pool(name="persist", bufs=1) as pp,
        ):
            ident_f32 = const_pool.tile([128, 128], F32)
            ident_f16 = const_pool.tile([128, 128], F16)
            masks.make_identity(nc, ident_f32[:])
            masks.make_identity(nc, ident_f16[:])
            gamma_bc = const_pool.tile([128, CH], F32)
            beta_bc = const_pool.tile([128, CH], F32)
            eps_t = const_pool.tile([128, 1], F32)
            nc.gpsimd.memset(eps_t[:], EPS)
            nc.sync.dma_start(gamma_bc[:], gamma_d[:].broadcast_to((128, CH)))
            nc.sync.dma_start(beta_bc[:], beta_d[:].broadcast_to((128, CH)))

            # ---- persistent SBUF tensors (fp16 for PE streaming) ----
            xT = [pp.tile([128, s], F16, tag=f"xT{ec}", name=f"xT{ec}")
                  for ec in range(4)]
            wT = {
                w: [pp.tile([128, CH], F16, tag=f"{w}T{ec}", name=f"{w}T{ec}")
                    for ec in range(4)]
                for w in ("wq", "wk", "wv")
            }
            # ErT replicated into both partition halves so each head of a
            # row-tiled pair finds it at its own base partition.
            erT = pp.tile([128, s], F16, tag="erT")
            qT = [pp.tile([128, s], F16, tag=f"qT{oc}", name=f"qT{oc}")
                  for oc in range(NPAIR)]
            kT = [pp.tile([128, s], F16, tag=f"kT{oc}", name=f"kT{oc}")
                  for oc in range(NPAIR)]
            # v with a ones column appended per head: [128, HLOC*(HD+1)] fp16
            vaug = [pp.tile([128, HLOC * (HD + 1)], F16, tag=f"va{sb}",
                            name=f"va{sb}") for sb in range(SB)]
            outp = [pp.tile([128, CH], F32, tag=f"op{sb}", name=f"op{sb}")
                    for sb in range(SB)]

            # ================= setup: load + transpose =================
            with (
                tc.tile_pool(name="ld", bufs=4) as ld_pool,
                tc.tile_pool(name="ps_set", bufs=4, space="PSUM") as ps_set,
            ):
                # Warm-up: absorb the Pool (identity-creation) dependency
                # into PE's observed clock.
                warm = ps_set.tile([128, 128], F32, tag="pset")
                nc.tensor.matmul(
                    warm[:], ident_f32[:], ident_f32[:], start=True, stop=True)

                # xT[ec][:, i*128:(i+1)*128] = x[i-block, ec-block].T
                for sb in range(SB):
                    xt = ld_pool.tile([128, E], F32, tag="xld")
                    nc.sync.dma_start(xt[:], x_d[sb * 128:(sb + 1) * 128, :])
                    for ec in range(4):
                        pst = ps_set.tile([128, 128], F32, tag="pset")
                        nc.tensor.transpose(
                            pst[:], xt[:, ec * 128:(ec + 1) * 128],
                            ident_f32[:])
                        if (sb + ec) % 2 == 0:
                            nc.vector.tensor_copy(
                                xT[ec][:, sb * 128:(sb + 1) * 128], pst[:])
                        else:
                            nc.scalar.copy(
                                xT[ec][:, sb * 128:(sb + 1) * 128], pst[:])
                # weights
                for w_name, w_d in (("wq", wq_d), ("wk", wk_d), ("wv", wv_d)):
                    for pc in range(CH // 128):
                        wt = ld_pool.tile([128, E], F32, tag="wld")
                        nc.sync.dma_start(
                            wt[:], w_d[pc * 128:(pc + 1) * 128, :])
                        for ec in range(4):
                            pst = ps_set.tile([128, 128], F32, tag="pset")
                            nc.tensor.transpose(
                                pst[:], wt[:, ec * 128:(ec + 1) * 128],
                                ident_f32[:])
                            if (pc + ec) % 2 == 0:
                                nc.scalar.copy(
                                    wT[w_name][ec][:, pc * 128:(pc + 1) * 128],
                                    pst[:])
                            else:
                                nc.vector.tensor_copy(
                                    wT[w_name][ec][:, pc * 128:(pc + 1) * 128],
                                    pst[:])
                # Er: transpose into both partition halves, one DVE copy
                for sb in range(SB):
                    et = ld_pool.tile([128, HD], F32, tag="eld")
                    nc.sync.dma_start(et[:], er_d[sb * 128:(sb + 1) * 128, :])
                    pst = ps_set.tile([128, 128], F32, tag="psete")
                    nc.tensor.transpose(pst[0:64, :], et[:], ident_f32[:])
                    nc.tensor.matmul(
                        pst[64:128, :], et[:], ident_f32[:],
                        start=True, stop=True)
                    nc.vector.tensor_copy(
                        erT[:, sb * 128:(sb + 1) * 128], pst[:])

            # ================= projections =================
            with (
                tc.tile_pool(name="wrk", bufs=4) as wrk,
                tc.tile_pool(name="wrk2", bufs=3) as wrk2,
                tc.tile_pool(name="wrk3", bufs=5) as wrk3,
                tc.tile_pool(name="pex", bufs=3) as pex,
                tc.tile_pool(name="srl", bufs=3) as srl,
                tc.tile_pool(name="small", bufs=8) as small,
            ):
                with tc.tile_pool(
                        name="ps_pj", bufs=4, space="PSUM") as ps_pj:
                    # qT / kT: [oc*128+p, t] = sum_e W[oc*128+p, e] x[t, e]
                    for dst, w_name in ((qT, "wq"), (kT, "wk")):
                        for oc in range(NPAIR):
                            for sc in range(KC):
                                ps = ps_pj.tile([128, 512], F32, tag="pj")
                                for ec in range(4):
                                    nc.tensor.matmul(
                                        ps[:],
                                        wT[w_name][ec][:, oc * 128:
                                                       (oc + 1) * 128],
                                        xT[ec][:, sc * 512:(sc + 1) * 512],
                                        start=(ec == 0), stop=(ec == 3))
                                if (oc + sc) % 2 == 0:
                                    nc.scalar.copy(
                                        dst[oc][:, sc * 512:(sc + 1) * 512],
                                        ps[:])
                                else:
                                    nc.vector.tensor_copy(
                                        dst[oc][:, sc * 512:(sc + 1) * 512],
                                        ps[:])
                    # v natural + ones column, fp16; one strided copy per
                    # block moves all 4 heads, one strided memset the ones.
                    for sb in range(SB):
                        ps = ps_pj.tile([128, CH], F32, tag="pj")
                        for ec in range(4):
                            nc.tensor.matmul(
                                ps[:],
                                xT[ec][:, sb * 128:(sb + 1) * 128],
                                wT["wv"][ec][:],
                                start=(ec == 0), stop=(ec == 3))
                        va = vaug[sb][:].rearrange(
                            "p (h d) -> p h d", h=HLOC)
                        nc.vector.tensor_copy(
                            va[:, :, 0:HD],
                            ps[:].rearrange("p (h d) -> p h d", h=HLOC))
                        nc.vector.memset(va[:, :, HD:HD + 1], 1.0)

                # ---------------- per-head-pair attention ----------------
                from contextlib import ExitStack
                att_stk = ExitStack()
                ps_half = att_stk.enter_context(tc.tile_pool(
                    name="ps_half", bufs=2, space="PSUM"))
                ps_tr = att_stk.enter_context(tc.tile_pool(
                    name="ps_tr", bufs=2, space="PSUM"))
                ps_av = att_stk.enter_context(tc.tile_pool(
                    name="ps_av", bufs=2, space="PSUM"))

                # evac engine alternation: ~1 in 5 goes to ACT (the rest
                # to DVE) so both engines' totals balance given ACT also
                # owns the exps.
                evac_ctr = [0]

                def evac_copy(dst, src):
                    evac_ctr[0] += 1
                    if evac_ctr[0] % 5 == 0:
                        nc.scalar.copy(dst, src)
                    else:
                        nc.vector.tensor_copy(dst, src)

                def phase_a_prep(p, sb):
                    """Allocate padded-P fp8 tiles for heads (2p, 2p+1)."""
                    pexpA = pex.tile([128, s + 1], FP8, tag="pexA",
                                     name="pexpA")
                    pexpB = pex.tile([128, s + 1], FP8, tag="pexB",
                                     name="pexpB")
                    nc.vector.memset(pexpA[:, 0:1], 0.0)
                    nc.vector.memset(pexpB[:, 0:1], 0.0)
                    return (pexpA, pexpB)

                def phase_a_chunk(p, sb, pexps, c):
                    """QEr chunk c for the row-tiled head pair + evac."""
                    qcols = (sb * 128, (sb + 1) * 128)
                    c0 = c * CW
                    psA = ps_half.tile([128, CW], F32, tag="ph", name="psA")
                    psB = ps_half.tile([128, CW], F32, tag="ph", name="psB")
                    for w in range(CW // 512):
                        w0 = c0 + w * 512
                        nc.tensor.matmul(
                            psA[:, w * 512:(w + 1) * 512],
                            qT[p][0:64, qcols[0]:qcols[1]],
                            erT[0:64, w0:w0 + 512],
                            start=True, stop=True)
                        nc.tensor.matmul(
                            psB[:, w * 512:(w + 1) * 512],
                            qT[p][64:128, qcols[0]:qcols[1]],
                            erT[64:128, w0:w0 + 512],
                            start=True, stop=True)
                    evac_copy(pexps[0][:, 1 + c0:1 + c0 + CW], psA[:])
                    evac_copy(pexps[1][:, 1 + c0:1 + c0 + CW], psB[:])

                def phase_a_finish(p, sb, pexps):
                    for h, pexp in ((2 * p, pexps[0]), (2 * p + 1, pexps[1])):
                        nc.sync.dma_start(
                            p_d[h, sb * 128 * (s + 1):
                                (sb * 128 + 128) * (s + 1)]
                            .rearrange("(r c) -> r c", c=s + 1),
                            pexp[:])

                def phase_a_pair(p, sb):
                    pexps = phase_a_prep(p, sb)
                    for c in range(NCH):
                        phase_a_chunk(p, sb, pexps, c)
                    phase_a_finish(p, sb, pexps)

                def phase_b_front(p, sb):
                    """QK matmuls + srel add for the pair -> fp16 scores.

                    With DMA_ACCUM_SREL the QK psum is cast-evacuated to
                    sc_t and the skewed fp8 srel is added in the SDMA
                    datapath (SWDGE accumulate DMA, no engine cost);
                    otherwise DVE tensor_adds an SBUF fp8 srel tile."""
                    hA, hB = 2 * p, 2 * p + 1
                    qcols = (sb * 128, (sb + 1) * 128)
                    base = (sb * 128 + 1) * s
                    srels = []
                    if not DMA_ACCUM_SREL:
                        for h in (hA, hB):
                            srel = srl.tile([128, s], FP8, tag=f"sr{h % 2}",
                                            name="srel")
                            nc.sync.dma_start(
                                srel[:],
                                p_d[h, base:base + 128 * s]
                                .rearrange("(r c) -> r c", c=s))
                            srels.append(srel)
                    sc_ts = []
                    for hi in range(2):
                        sc_t = wrk2.tile([128, s], F16, tag=f"sc{hi}",
                                         name="sc_t")
                        sc_ts.append(sc_t)
                    for c in range(NCH):
                        c0 = c * CW
                        psA = ps_half.tile([128, CW], F32, tag="ph",
                                           name="psQA")
                        psB = ps_half.tile([128, CW], F32, tag="ph",
                                           name="psQB")
                        for w in range(CW // 512):
                            w0 = c0 + w * 512
                            nc.tensor.matmul(
                                psA[:, w * 512:(w + 1) * 512],
                                qT[p][0:64, qcols[0]:qcols[1]],
                                kT[p][0:64, w0:w0 + 512],
                                start=True, stop=True)
                            nc.tensor.matmul(
                                psB[:, w * 512:(w + 1) * 512],
                                qT[p][64:128, qcols[0]:qcols[1]],
                                kT[p][64:128, w0:w0 + 512],
                                start=True, stop=True)
                        if DMA_ACCUM_SREL:
                            for hi, ps in ((0, psA), (1, psB)):
                                evac_copy(sc_ts[hi][:, c0:c0 + CW], ps[:])
                                nc.gpsimd.dma_start(
                                    sc_ts[hi][:, c0:c0 + CW],
                                    p_d[(hA, hB)[hi], base:base + 128 * s]
                                    .rearrange("(r c) -> r c", c=s)
                                    [:, c0:c0 + CW],
                                    accum_op=mybir.AluOpType.add)
                        else:
                            nc.vector.tensor_add(
                                sc_ts[0][:, c0:c0 + CW], psA[:],
                                srels[0][:, c0:c0 + CW])
                            nc.vector.tensor_add(
                                sc_ts[1][:, c0:c0 + CW], psB[:],
                                srels[1][:, c0:c0 + CW])
                    return sc_ts

                def phase_b_texp(p, sb, sc_ts, fillers):
                    """transpose+exp for both heads.  `fillers` are
                    emitted between transpose blocks: transpose-mode
                    matmuls do not register as PE activity for the HAM
                    clock gate, so real matmuls (next pair's QEr, the
                    previous block's AV) are interleaved to keep the PE
                    clock at 8/8."""
                    tpss = []
                    fi = 0
                    for hi in range(2):
                        for half in range(NHALF):
                            pst = ps_tr.tile([128, CW], F16, tag="tr",
                                             name="pst")
                            for t in range(NTH2):
                                c0 = half * CW + t * 128
                                nc.tensor.transpose(
                                    pst[:, t * 128:(t + 1) * 128],
                                    sc_ts[hi][:, c0:c0 + 128],
                                    ident_f16[:])
                            tps = wrk3.tile([128, CW], F16, tag=f"tps{hi}",
                                            name="tps")
                            nc.scalar.activation(
                                tps[:], pst[:],
                                mybir.ActivationFunctionType.Exp, scale=SCALE)
                            tpss.append(tps)
                            if fi < len(fillers):
                                fillers[fi]()
                                fi += 1
                    for f in fillers[fi:]:
                        f()
                    return tpss

                def phase_b_av(p, sb, tpss):
                    """AV matmuls + normalize; runs one block behind
                    texp so the exps have long since drained and the AV
                    matmuls never stall the PE queue."""
                    for hi, h in enumerate((2 * p, 2 * p + 1)):
                        pc_av = ps_av.tile([128, HD + 1], F32, tag="av",
                                           name="pc_av")
                        for half in range(NHALF):
                            tps = tpss[hi * NHALF + half]
                            for t in range(NTH2):
                                ci = half * NTH2 + t
                                nc.tensor.matmul(
                                    pc_av[:],
                                    tps[:, t * 128:(t + 1) * 128],
                                    vaug[ci][:, (h % HLOC) * (HD + 1):
                                             (h % HLOC + 1) * (HD + 1)],
                                    start=(ci == 0), stop=(ci == NTH - 1))
                        rinv = small.tile([128, 1], F32, tag="rinv",
                                          name="rinv")
                        nc.vector.reciprocal(rinv[:], pc_av[:, HD:HD + 1])
                        nc.vector.tensor_scalar_mul(
                            outp[sb][:, (h % HLOC) * HD:(h % HLOC + 1) * HD],
                            pc_av[:, 0:HD], rinv[:])

                def ln_stats_block(sb):
                    s1 = small.tile([128, 1], F32, tag="s1", name="s1")
                    nc.vector.reduce_sum(
                        s1[:], outp[sb][:], axis=mybir.AxisListType.X)
                    sq = small.tile([128, 1], F32, tag="sq", name="sq")
                    scr = wrk.tile([128, CH], F32, tag="scr", name="scr")
                    nc.scalar.activation(
                        scr[:], outp[sb][:],
                        mybir.ActivationFunctionType.Square, accum_out=sq[:])
                    nc.sync.dma_start(
                        cc_in[sb * 128:(sb + 1) * 128, 0:1], s1[:])
                    nc.sync.dma_start(
                        cc_in[sb * 128:(sb + 1) * 128, 1:2], sq[:])

                # software pipeline: srel(p, sb) only reads pexp blocks
                # sb and sb+1, so phase A runs a rolling LAG blocks
                # ahead of phase B.  Per iteration the emission order is
                #   [QK matmuls] [T+exp blocks with interleaved fillers]
                # where the fillers are the next phase-A QEr chunks and
                # the PREVIOUS iteration's AV matmuls -- real matmuls
                # between the HAM-invisible transpose stretches, and the
                # AVs trail one iteration so their exps are long done.
                LAG = min(3, SB)
                for sb in range(LAG):
                    phase_a_pair(0, sb)
                carry = None  # (pr, sb, tpss) awaiting AV
                for pr in range(NPAIR):
                    for sb in range(SB):
                        sc_ts = phase_b_front(pr, sb)
                        na, npr = sb + LAG, pr
                        if na >= SB:
                            na -= SB
                            npr += 1
                        fillers = []
                        if npr < NPAIR:
                            pexps = phase_a_prep(npr, na)
                            for c in range(NCH):
                                fillers.append(
                                    lambda npr=npr, na=na, pexps=pexps, c=c:
                                    phase_a_chunk(npr, na, pexps, c))
                        if carry is not None:
                            cpr, csb, ctpss = carry
                            fillers.append(
                                lambda cpr=cpr, csb=csb, ctpss=ctpss:
                                phase_b_av(cpr, csb, ctpss))
                            if cpr + 1 == NPAIR:
                                fillers.append(
                                    lambda csb=csb: ln_stats_block(csb))
                        tpss = phase_b_texp(p=pr, sb=sb, sc_ts=sc_ts,
                                            fillers=fillers)
                        if npr < NPAIR:
                            phase_a_finish(npr, na, pexps)
                        carry = (pr, sb, tpss)
                # drain the last AV
                cpr, csb, ctpss = carry
                phase_b_av(cpr, csb, ctpss)
                ln_stats_block(csb)

                att_stk.close()
                # ================= LayerNorm =================
                nc.gpsimd.collective_compute(
                    "AllReduce", mybir.AluOpType.add,
                    replica_groups=pairs,
                    ins=[cc_in[:].opt()], outs=[cc_out[:].opt()])
                for sb in range(SB):
                    st = small.tile([128, 2], F32, tag="st")
                    nc.sync.dma_start(st[:], cc_out[sb * 128:(sb + 1) * 128, :])
                    mean = small.tile([128, 1], F32, tag="mean")
                    nc.vector.tensor_scalar_mul(mean[:], st[:, 0:1], 1.0 / E)
                    ex2 = small.tile([128, 1], F32, tag="ex2")
                    nc.vector.tensor_scalar_mul(ex2[:], st[:, 1:2], 1.0 / E)
                    msq = small.tile([128, 1], F32, tag="msq")
                    nc.vector.tensor_mul(msq[:], mean[:], mean[:])
                    var = small.tile([128, 1], F32, tag="var")
                    nc.vector.tensor_sub(var[:], ex2[:], msq[:])
                    std = small.tile([128, 1], F32, tag="std")
                    nc.scalar.activation(
                        std[:], var[:],
                        mybir.ActivationFunctionType.Sqrt, bias=eps_t[:])
                    rstd = small.tile([128, 1], F32, tag="rstd")
                    nc.vector.reciprocal(rstd[:], std[:])
                    tmp = wrk.tile([128, CH], F32, tag="tmp")
                    nc.vector.tensor_scalar(
                        tmp[:], outp[sb][:], mean[:], rstd[:],
                        op0=mybir.AluOpType.subtract,
                        op1=mybir.AluOpType.mult)
                    y1 = wrk2.tile([128, CH], F32, tag="y1")
                    nc.vector.tensor_mul(y1[:], tmp[:], gamma_bc[:])
                    y2 = wrk3.tile([128, CH], F32, tag="y2")
                    nc.vector.tensor_add(y2[:], y1[:], beta_bc[:])
                    nc.sync.dma_start(out_d[sb * 128:(sb + 1) * 128, :], y2[:])

    if legalize:
        _legalize_waits(nc)
    return nc


def _legalize_waits(nc):
    """walrus's codegen accepts at most one sync wait on most instruction
    structs; hoist extra waits onto NoOps inserted just before, on the
    same engine queue (program order preserves the semantics)."""
    n = 0
    keep = set()
    for bb in nc.main_func.blocks:
        out = []
        for inst in bb.instructions:
            si = inst.sync_info
            if (inst.opcode not in keep and si is not None
                    and si.on_wait and len(si.on_wait) > 1):
                for w in si.on_wait[:-1]:
                    nop = mybir.InstNoOp(
                        name=f"I-mmw{n}", ins=[], outs=[])
                    n += 1
                    nop.engine = inst.engine
                    nop.sync_info = mybir.SyncInfo(
                        on_wait=[w], on_update=[])
                    out.append(nop)
                si.on_wait = [si.on_wait[-1]]
            out.append(inst)
        bb.instructions = out
    return nc


_NC_CACHE = {}


def _get_nc(s=S, n_cores=N_CORES):
    key = (s, n_cores)
    if key not in _NC_CACHE:
        _NC_CACHE[key] = build_nc(s, n_cores)
    return _NC_CACHE[key]


def make_in_maps(x, Wq, Wk, Wv, Er, gamma, beta, n_cores=N_CORES):
    in_maps = []
    for c in range(n_cores):
        b, hg = c // 2, c % 2
        sl = slice(hg * CH, (hg + 1) * CH)
        in_maps.append({
            "x": np.ascontiguousarray(x[b], dtype=np.float32),
            "wq": np.ascontiguousarray(Wq[sl], dtype=np.float32),
            "wk": np.ascontiguousarray(Wk[sl], dtype=np.float32),
            "wv": np.ascontiguousarray(Wv[sl], dtype=np.float32),
            "er": np.ascontiguousarray(Er, dtype=np.float32),
            "gamma": np.ascontiguousarray(gamma[sl], dtype=np.float32)[None, :],
            "beta": np.ascontiguousarray(beta[sl], dtype=np.float32)[None, :],
        })
    return in_maps


def assemble(results, n_cores=N_CORES, s=S):
    y = np.empty((n_cores // 2, s, E), np.float32)
    for c in range(n_cores):
        y[c // 2, :, (c % 2) * CH:(c % 2 + 1) * CH] = results[c]["out"]
    return y


def kernel(**inputs):
    from concourse.bass_utils import run_bass_kernel_spmd
    nc = _get_nc()
    in_maps = make_in_maps(
        inputs["x"], inputs["Wq"], inputs["Wk"], inputs["Wv"],
        inputs["Er"], inputs["gamma"], inputs["beta"])
    res = run_bass_kernel_spmd(nc, in_maps, list(range(N_CORES)))
    return assemble(res.results)
